# revision 4
# baseline (speedup 1.0000x reference)
"""Multi-head transposed (channel) attention kernel for Trainium2.

Reference computation (per batch b, head h, c=32 channels, n=65536 spatial):
    q,k,v = split(qkv)                       # each [32, n] per (b,h)
    qh = q / max(||q||_row, 1e-12)           # L2 normalize over n
    kh = k / max(||k||_row, 1e-12)
    S = (qh @ kh.T) * temperature[h]         # [32, 32]
    A = softmax(S, axis=-1)
    out = A @ v                              # [32, n]

Sharding: 24 (b,h) pairs over 8 cores = 3 pairs/core, stacked on 96
partitions.  On the host, q and k are L2-normalized, temperature is folded
into q's rows, both are scaled by 64 and cast to fp8 e4m3 (values ~N(0,.25)
sit in e4m3's sweet spot); v is cast to fp16.  The output is produced in
fp16 and upcast on the host.

qk is pre-transposed on the host into the exact SBUF tile layout
[chunk, 128 (spatial), sub, 192 (q|k channels)], so pass-1 loads are fully
contiguous plain DMAs at HBM line rate.

Per core:
  pass 1: stream qk tiles; per 128-spatial sub ONE fp8 matmul accumulates
          S^T = k.T-chunk.T @ q.T-chunk into PSUM (96 moving cols, FWL
          weight loads hidden) - S lands already transposed for pass 2.
  chain:  exp fuses the 1/4096 descale and writes block-diagonal fp16
          attn^T directly from PSUM; softmax denominators via a
          ones-vector matmul, transposed back onto partitions.
  pass 2: out = attn^T-block-diag @ v in fp16 N=512 matmuls; PSUM->SBUF
          copies (with 1/rowsum scale) alternate between DVE and ACT.
          All of v is SBUF-resident; its 16 loads are posted on the sync
          ring right after the last qk load so DMA never idles across the
          pass-1 -> pass-2 transition.
"""

import ml_dtypes
import numpy as np

import concourse.bass as bass
import concourse.tile as tile
from concourse import bacc, mybir
from concourse.bass_utils import run_bass_kernel_spmd

F32 = mybir.dt.float32
F16 = mybir.dt.float16
F8 = mybir.dt.float8e4

B = 4
HD = 6
CH = 32          # channels per head
HW = 65536       # spatial size (256*256)
P = 96           # partition stack: 3 pairs * 32 channels
P2 = 192         # q-stack + k-stack channels
N_CORES = 8
PAIRS_PER_CORE = 3

QSCALE = 64.0    # host-side scale on normalized q and k before fp8 cast
DESCALE = 1.0 / (QSCALE * QSCALE)

FT = 4096        # pass-1 transpose-DMA chunk (spatial)
NCH1 = HW // FT  # 16
SUB = 128
NSUB = FT // SUB  # 32
F2 = 4096        # pass-2 v-load / out-store chunk
NF = 512         # matmul free size (one PSUM bank)
NMM2 = F2 // NF  # 8
NCH2 = HW // F2  # 16


def build_nc():
    nc = bacc.Bacc("TRN2", target_bir_lowering=False, debug=False,
                   num_devices=N_CORES)
    qk_d = nc.dram_tensor("qk", [NCH1, SUB, NSUB, P2], F8,
                          kind="ExternalInput").ap()
    v_d = nc.dram_tensor("v", [P, HW], F16, kind="ExternalInput").ap()
    o_d = nc.dram_tensor("out", [P, HW], F16, kind="ExternalOutput").ap()

    with tile.TileContext(nc) as tc:
        _body(nc, tc, qk_d, v_d, o_d)
    nc.compile()
    return nc


def _body(nc, tc, qk_d, v_d, o_d):
    Exp = mybir.ActivationFunctionType.Exp
    Copy = mybir.ActivationFunctionType.Copy

    with tc.tile_pool(name="persist", bufs=1) as pp:
        # warm the ACT Exp table off the critical path
        warm = pp.tile([1, 1], F32)
        nc.gpsimd.memset(warm[:, :], 1.0)
        nc.scalar.activation(out=warm[:, :], in_=warm[:, :], func=Exp)

        E_sb = pp.tile([P, P], F16)
        nc.gpsimd.memset(E_sb[:, :], 0.0)
        ones96 = pp.tile([P, 1], F16)
        nc.gpsimd.memset(ones96[:, :], 1.0)
        ident1 = pp.tile([1, 1], F32)
        nc.gpsimd.memset(ident1[:, :], 1.0)
        rs_sb = pp.tile([1, P], F32)
        rinv = pp.tile([P, 1], F32)

        # one PSUM bank accumulates S^T = kT.T @ qT over all 512 subs
        psS_cm = tc.tile_pool(name="psS", bufs=1, space="PSUM")
        psS_p = psS_cm.__enter__()
        acc = psS_p.tile([P, P], F32)

        # v is fully SBUF-resident; allocate all 16 tiles up front so the
        # loads can be posted with no buffer-recycle waits
        iov = tc.tile_pool(name="iov", bufs=1)
        iov_p = iov.__enter__()
        v_tiles = [iov_p.tile([P, F2], F16, tag=f"v{t}", name=f"v{t}")
                   for t in range(NCH2)]

        # ---------------- pass 1: S^T ----------------
        with tc.tile_pool(name="io1", bufs=6) as io1:
            for t in range(NCH1):
                qkT = io1.tile([SUB, NSUB, P2], F8, tag="qkT")
                nc.sync.dma_start(out=qkT[:, :, :], in_=qk_d[t])
                for s in range(NSUB):
                    first = (t == 0 and s == 0)
                    last = (t == NCH1 - 1 and s == NSUB - 1)
                    nc.tensor.matmul(
                        acc[:, :],
                        lhsT=qkT[:, s, P:P2],
                        rhs=qkT[:, s, 0:P],
                        start=first, stop=last, skip_group_check=True)

        # post all v loads now: they queue on the sync ring behind the
        # final qk loads, keeping the DMA engines saturated through the
        # softmax chain and into pass 2
        for t in range(NCH2):
            sl = slice(t * F2, (t + 1) * F2)
            nc.sync.dma_start(out=v_tiles[t][:, :], in_=v_d[:, sl])

        # ---------------- softmax chain ----------------
        with tc.tile_pool(name="psC", bufs=1, space="PSUM") as psC:
            # block-diagonal unnormalized attn^T in fp16, straight from PSUM
            for j in range(PAIRS_PER_CORE):
                blk = slice(CH * j, CH * (j + 1))
                nc.scalar.activation(out=E_sb[blk, blk], in_=acc[blk, blk],
                                     func=Exp, scale=DESCALE)
            # softmax denominators: column sums of E via ones-matmul,
            # transposed back onto partitions
            rs_ps = psC.tile([1, P], F32, tag="rs")
            nc.tensor.matmul(rs_ps[:, :], lhsT=ones96[:, :], rhs=E_sb[:, :],
                             start=True, stop=True)
            nc.vector.tensor_copy(out=rs_sb[:, :], in_=rs_ps[:, :])
            ri_ps = psC.tile([P, 1], F32, tag="ri")
            nc.tensor.transpose(ri_ps[:, :], rs_sb[:, :], ident1[:, :])
            nc.vector.reciprocal(out=rinv[:, :], in_=ri_ps[:, :])

        # release the accumulator bank so pass 2 can use 8 PSUM banks
        psS_cm.__exit__(None, None, None)

        # ---------------- pass 2: out = attn @ v ----------------
        with (
            tc.tile_pool(name="ioo", bufs=4) as ioo,
            tc.tile_pool(name="psO", bufs=8, space="PSUM") as psOp,
        ):
            mult = mybir.AluOpType.mult
            for t in range(NCH2):
                sl = slice(t * F2, (t + 1) * F2)
                on = ioo.tile([P, F2], F16, tag="on")
                for m in range(NMM2):
                    msl = slice(m * NF, (m + 1) * NF)
                    o_ps = psOp.tile([P, NF], F32, tag="o")
                    nc.tensor.matmul(o_ps[:, :], lhsT=E_sb[:, :],
                                     rhs=v_tiles[t][:, msl],
                                     start=True, stop=True)
                    if (NMM2 * t + m) % 2 == 0:
                        nc.vector.tensor_scalar(
                            out=on[:, msl], in0=o_ps[:, :],
                            scalar1=rinv[:, :], scalar2=None, op0=mult)
                    else:
                        nc.scalar.activation(out=on[:, msl], in_=o_ps[:, :],
                                             func=Copy, scale=rinv[:, :])
                nc.scalar.dma_start(out=o_d[:, sl], in_=on[:, :])

        iov.__exit__(None, None, None)


_NC_CACHE = {}


def _get_nc():
    if "nc" not in _NC_CACHE:
        _NC_CACHE["nc"] = build_nc()
    return _NC_CACHE["nc"]


def _shard_inputs(qkv, temperature):
    qkv = np.asarray(qkv)
    temp = np.asarray(temperature, dtype=np.float32).reshape(-1)
    C = HD * CH
    q = qkv[:, 0 * C:1 * C].reshape(B, HD, CH, HW)
    k = qkv[:, 1 * C:2 * C].reshape(B, HD, CH, HW)
    v = qkv[:, 2 * C:3 * C].reshape(B, HD, CH, HW)

    # fold L2 normalization, temperature, and the fp8 range scale into the
    # host-side quantization of q and k
    qn = np.maximum(np.sqrt(np.einsum('bhcn,bhcn->bhc', q, q)), 1e-12)
    kn = np.maximum(np.sqrt(np.einsum('bhcn,bhcn->bhc', k, k)), 1e-12)
    qs = (QSCALE * temp[None, :, None] / qn)[..., None]
    ks = (QSCALE / kn)[..., None]
    q8 = (q * qs).astype(ml_dtypes.float8_e4m3)
    k8 = (k * ks).astype(ml_dtypes.float8_e4m3)

    in_maps = []
    for core in range(N_CORES):
        pairs = [divmod(p, HD) for p in
                 range(core * PAIRS_PER_CORE, (core + 1) * PAIRS_PER_CORE)]
        qs_ = np.concatenate([q8[b_, h_] for b_, h_ in pairs], axis=0)
        ks_ = np.concatenate([k8[b_, h_] for b_, h_ in pairs], axis=0)
        qks = np.concatenate([qs_, ks_], axis=0)
        # pre-transpose to the SBUF tile layout [chunk, p, sub, ch]
        qks = np.ascontiguousarray(
            qks.reshape(P2, NCH1, NSUB, SUB).transpose(1, 3, 2, 0))
        vs = np.concatenate([v[b_, h_] for b_, h_ in pairs],
                            axis=0).astype(np.float16)
        in_maps.append({"qk": qks, "v": vs})
    return in_maps


def _gather_output(results):
    out = np.empty((B, HD, CH, HW), dtype=np.float32)
    for core in range(N_CORES):
        o = results[core]["out"]
        for j in range(PAIRS_PER_CORE):
            b_, h_ = divmod(core * PAIRS_PER_CORE + j, HD)
            out[b_, h_] = o[CH * j:CH * (j + 1)].astype(np.float32)
    return out.reshape(B, HD * CH, 256, 256)


def kernel(qkv, temperature):
    in_maps = _shard_inputs(qkv, temperature)
    nc = _get_nc()
    res = run_bass_kernel_spmd(nc, in_maps, list(range(N_CORES)))
    return _gather_output(res.results)


if __name__ == "__main__":
    rng = np.random.default_rng(0)
    qkv = rng.standard_normal((B, 576, 256, 256), dtype=np.float32)
    temp = np.ones((HD, 1, 1), dtype=np.float32)
    out = kernel(qkv=qkv, temperature=temp)
    print("out", out.shape, out.dtype, float(np.abs(out).max()))


# revision 10
# speedup vs baseline: 1.1864x; 1.1864x over previous
"""Multi-head transposed (channel) attention kernel for Trainium2.

Reference computation (per batch b, head h, c=32 channels, n=65536 spatial):
    q,k,v = split(qkv)                       # each [32, n] per (b,h)
    qh = q / max(||q||_row, 1e-12)           # L2 normalize over n
    kh = k / max(||k||_row, 1e-12)
    S = (qh @ kh.T) * temperature[h]         # [32, 32]
    A = softmax(S, axis=-1)
    out = A @ v                              # [32, n]

Sharding: 24 (b,h) pairs over 8 cores = 3 pairs/core, stacked on 96
partitions.  On the host, q and k are L2-normalized, temperature is folded
into q's rows, both are scaled by 64 and cast to fp8 e4m3 (values ~N(0,.25)
sit in e4m3's sweet spot); v is cast to fp16.  The output is produced in
fp16 and upcast on the host.

qk is pre-transposed on the host into the exact SBUF tile layout
[chunk, 128 (spatial), sub, 192 (q|k channels)], so pass-1 loads are fully
contiguous plain DMAs at HBM line rate.

Per core:
  pass 1: stream qk tiles; per 128-spatial sub ONE fp8 matmul accumulates
          S^T = k.T-chunk.T @ q.T-chunk into PSUM (96 moving cols, FWL
          weight loads hidden) - S lands already transposed for pass 2.
  chain:  exp fuses the 1/4096 descale and writes block-diagonal fp16
          attn^T directly from PSUM; softmax denominators via a
          ones-vector matmul, transposed back onto partitions.
  pass 2: out = attn^T-block-diag @ v in fp16 N=512 matmuls; PSUM->SBUF
          copies (with 1/rowsum scale) alternate between DVE and ACT.
          All of v is SBUF-resident; its 16 loads are posted on the sync
          ring right after the last qk load so DMA never idles across the
          pass-1 -> pass-2 transition.
"""

import ml_dtypes
import numpy as np

import concourse.bass as bass
import concourse.tile as tile
from concourse import bacc, mybir
from concourse.bass_utils import run_bass_kernel_spmd

F32 = mybir.dt.float32
F16 = mybir.dt.float16
F8 = mybir.dt.float8e4

B = 4
HD = 6
CH = 32          # channels per head
HW = 65536       # spatial size (256*256)
P = 96           # partition stack: 3 pairs * 32 channels
P2 = 192         # q-stack + k-stack channels
N_CORES = 8
PAIRS_PER_CORE = 3

QSCALE = 64.0    # host-side scale on normalized q and k before fp8 cast
DESCALE = 1.0 / (QSCALE * QSCALE)

FT = 4096        # pass-1 transpose-DMA chunk (spatial)
NCH1 = HW // FT  # 16
SUB = 128
NSUB = FT // SUB  # 32
F2 = 4096        # pass-2 v-load / out-store chunk
NF = 512         # matmul free size (one PSUM bank)
NMM2 = F2 // NF  # 8
NCH2 = HW // F2  # 16


def build_nc():
    nc = bacc.Bacc("TRN2", target_bir_lowering=False, debug=False,
                   num_devices=N_CORES)
    qk_d = nc.dram_tensor("qk", [NCH1, SUB, NSUB, P2], F8,
                          kind="ExternalInput").ap()
    # v and out are chunk-major [chunk, 96, F2] so every DMA walks a single
    # contiguous 768KB HBM block (sequential descriptors stream ~26GB/s per
    # engine; the strided [96, HW] walk measured ~40% slower)
    v_d = nc.dram_tensor("v", [NCH2, P, F2], F16, kind="ExternalInput").ap()
    o_d = nc.dram_tensor("out", [NCH2, P, F2], F16,
                         kind="ExternalOutput").ap()

    with tile.TileContext(nc) as tc:
        _body(nc, tc, qk_d, v_d, o_d)
    nc.compile()
    return nc


def _body(nc, tc, qk_d, v_d, o_d):
    Exp = mybir.ActivationFunctionType.Exp
    Copy = mybir.ActivationFunctionType.Copy

    with tc.tile_pool(name="persist", bufs=1) as pp:
        # warm the ACT Exp table off the critical path
        warm = pp.tile([1, 1], F32)
        nc.gpsimd.memset(warm[:, :], 1.0)
        nc.scalar.activation(out=warm[:, :], in_=warm[:, :], func=Exp)

        E_sb = pp.tile([P, P], F16)
        nc.gpsimd.memset(E_sb[:, :], 0.0)
        ones96 = pp.tile([P, 1], F16)
        nc.gpsimd.memset(ones96[:, :], 1.0)
        ident1 = pp.tile([1, 1], F32)
        nc.gpsimd.memset(ident1[:, :], 1.0)
        rs_sb = pp.tile([1, P], F32)
        rinv = pp.tile([P, 1], F32)

        # one PSUM bank accumulates S^T = kT.T @ qT over all 512 subs
        psS_cm = tc.tile_pool(name="psS", bufs=1, space="PSUM")
        psS_p = psS_cm.__enter__()
        acc = psS_p.tile([P, P], F32)

        # v is fully SBUF-resident; allocate all 16 tiles up front so the
        # loads can be posted with no buffer-recycle waits
        iov = tc.tile_pool(name="iov", bufs=1)
        iov_p = iov.__enter__()
        v_tiles = [iov_p.tile([P, F2], F16, tag=f"v{t}", name=f"v{t}")
                   for t in range(NCH2)]

        # ---------------- pass 1: S^T ----------------
        with tc.tile_pool(name="io1", bufs=6) as io1:
            for t in range(NCH1):
                qkT = io1.tile([SUB, NSUB, P2], F8, tag="qkT")
                nc.sync.dma_start(out=qkT[:, :, :], in_=qk_d[t])
                for s in range(NSUB):
                    first = (t == 0 and s == 0)
                    last = (t == NCH1 - 1 and s == NSUB - 1)
                    nc.tensor.matmul(
                        acc[:, :],
                        lhsT=qkT[:, s, P:P2],
                        rhs=qkT[:, s, 0:P],
                        start=first, stop=last, skip_group_check=True)

        # post all v loads now: they queue on the sync ring behind the
        # final qk loads, keeping the DMA engines saturated through the
        # softmax chain and into pass 2
        for t in range(NCH2):
            nc.sync.dma_start(out=v_tiles[t][:, :], in_=v_d[t])

        # ---------------- softmax chain ----------------
        with tc.tile_pool(name="psC", bufs=1, space="PSUM") as psC:
            # block-diagonal unnormalized attn^T in fp16, straight from PSUM
            for j in range(PAIRS_PER_CORE):
                blk = slice(CH * j, CH * (j + 1))
                nc.scalar.activation(out=E_sb[blk, blk], in_=acc[blk, blk],
                                     func=Exp, scale=DESCALE)
            # softmax denominators: column sums of E via ones-matmul,
            # transposed back onto partitions
            rs_ps = psC.tile([1, P], F32, tag="rs")
            nc.tensor.matmul(rs_ps[:, :], lhsT=ones96[:, :], rhs=E_sb[:, :],
                             start=True, stop=True)
            nc.vector.tensor_copy(out=rs_sb[:, :], in_=rs_ps[:, :])
            ri_ps = psC.tile([P, 1], F32, tag="ri")
            nc.tensor.transpose(ri_ps[:, :], rs_sb[:, :], ident1[:, :])
            nc.vector.reciprocal(out=rinv[:, :], in_=ri_ps[:, :])

        # release the accumulator bank so pass 2 can use 8 PSUM banks
        psS_cm.__exit__(None, None, None)

        # ---------------- pass 2: out = attn @ v ----------------
        with (
            tc.tile_pool(name="ioo", bufs=4) as ioo,
            tc.tile_pool(name="psO", bufs=8, space="PSUM") as psOp,
        ):
            mult = mybir.AluOpType.mult
            for t in range(NCH2):
                on = ioo.tile([P, F2], F16, tag="on")
                for m in range(NMM2):
                    msl = slice(m * NF, (m + 1) * NF)
                    o_ps = psOp.tile([P, NF], F32, tag="o")
                    nc.tensor.matmul(o_ps[:, :], lhsT=E_sb[:, :],
                                     rhs=v_tiles[t][:, msl],
                                     start=True, stop=True)
                    if (NMM2 * t + m) % 2 == 0:
                        nc.vector.tensor_scalar(
                            out=on[:, msl], in0=o_ps[:, :],
                            scalar1=rinv[:, :], scalar2=None, op0=mult)
                    else:
                        nc.scalar.activation(out=on[:, msl], in_=o_ps[:, :],
                                             func=Copy, scale=rinv[:, :])
                nc.scalar.dma_start(out=o_d[t], in_=on[:, :])

        iov.__exit__(None, None, None)


_NC_CACHE = {}


def _get_nc():
    if "nc" not in _NC_CACHE:
        _NC_CACHE["nc"] = build_nc()
    return _NC_CACHE["nc"]


def _shard_inputs(qkv, temperature):
    qkv = np.asarray(qkv)
    temp = np.asarray(temperature, dtype=np.float32).reshape(-1)
    C = HD * CH
    q = qkv[:, 0 * C:1 * C].reshape(B, HD, CH, HW)
    k = qkv[:, 1 * C:2 * C].reshape(B, HD, CH, HW)
    v = qkv[:, 2 * C:3 * C].reshape(B, HD, CH, HW)

    # fold L2 normalization, temperature, and the fp8 range scale into the
    # host-side quantization of q and k
    qn = np.maximum(np.sqrt(np.einsum('bhcn,bhcn->bhc', q, q)), 1e-12)
    kn = np.maximum(np.sqrt(np.einsum('bhcn,bhcn->bhc', k, k)), 1e-12)
    qs = (QSCALE * temp[None, :, None] / qn)[..., None]
    ks = (QSCALE / kn)[..., None]
    q8 = (q * qs).astype(ml_dtypes.float8_e4m3)
    k8 = (k * ks).astype(ml_dtypes.float8_e4m3)

    in_maps = []
    for core in range(N_CORES):
        pairs = [divmod(p, HD) for p in
                 range(core * PAIRS_PER_CORE, (core + 1) * PAIRS_PER_CORE)]
        qs_ = np.concatenate([q8[b_, h_] for b_, h_ in pairs], axis=0)
        ks_ = np.concatenate([k8[b_, h_] for b_, h_ in pairs], axis=0)
        qks = np.concatenate([qs_, ks_], axis=0)
        # pre-transpose to the SBUF tile layout [chunk, p, sub, ch]
        qks = np.ascontiguousarray(
            qks.reshape(P2, NCH1, NSUB, SUB).transpose(1, 3, 2, 0))
        vs = np.concatenate([v[b_, h_] for b_, h_ in pairs],
                            axis=0).astype(np.float16)
        # chunk-major [chunk, 96, F2] so each v DMA reads contiguous HBM
        vs = np.ascontiguousarray(
            vs.reshape(P, NCH2, F2).transpose(1, 0, 2))
        in_maps.append({"qk": qks, "v": vs})
    return in_maps


def _gather_output(results):
    out = np.empty((B, HD, CH, HW), dtype=np.float32)
    for core in range(N_CORES):
        # undo the chunk-major store layout [chunk, 96, F2] -> [96, HW]
        o = results[core]["out"].transpose(1, 0, 2).reshape(P, HW)
        for j in range(PAIRS_PER_CORE):
            b_, h_ = divmod(core * PAIRS_PER_CORE + j, HD)
            out[b_, h_] = o[CH * j:CH * (j + 1)].astype(np.float32)
    return out.reshape(B, HD * CH, 256, 256)


def kernel(qkv, temperature):
    in_maps = _shard_inputs(qkv, temperature)
    nc = _get_nc()
    res = run_bass_kernel_spmd(nc, in_maps, list(range(N_CORES)))
    return _gather_output(res.results)


if __name__ == "__main__":
    rng = np.random.default_rng(0)
    qkv = rng.standard_normal((B, 576, 256, 256), dtype=np.float32)
    temp = np.ones((HD, 1, 1), dtype=np.float32)
    out = kernel(qkv=qkv, temperature=temp)
    print("out", out.shape, out.dtype, float(np.abs(out).max()))
